# revision 13
# baseline (speedup 1.0000x reference)
"""Trainium2 Bass kernel for AttentionalAggregation-style GNN pooling.

reference math:
    enc  = relu(lane_encoding @ W.T + b)            # [M=400000, 512]
    maxp = segment_max(enc, seg)                    # [N=25000, 512], 16 lanes/group
    avgp = segment_mean(enc, seg)                   # [N=25000, 512]
    out  = concat([maxp, avgp], axis=1)             # [N, 1024]

Strategy (8 NeuronCores, data-parallel over lanes; each core owns whole groups):
  - Host pre-transposes x -> XT [128, M]; single-pass bf16 matmul (the 2e-2
    rel-err gate leaves ~5x margin over bf16 rounding noise).
  - relu(u + b) fused into the PSUM->SBUF evacuation, output bf16. The
    evacuation columns are split between the Scalar engine (activation with
    per-partition bias) and the Vector engine (tensor_scalar add-bias+max0),
    balancing the two engines.
  - max pool: windowed reduce over 16 contiguous bf16 lanes (relu and +b both
    commute with max, so no fixups). Split DVE / GPSIMD per span.
  - sum pool: mostly on the Tensor engine via 16 accumulating identity
    matmuls with stride-16 moving APs (PSUM accumulates the window sum);
    remainder on DVE/GPSIMD reduces. PE sums are deferred one block so the
    PE never stalls waiting on the evacuation of the block it just computed.
  - Pooled outputs stay transposed bf16 [512, G]; host converts to fp32 and
    applies the /16 mean divide.
"""
import sys

sys.path.insert(0, "/opt/trn_rl_repo")

import numpy as np
import ml_dtypes

import concourse.bass as bass
import concourse.bacc as bacc
import concourse.tile as tile
from concourse import mybir
from concourse.bass_utils import run_bass_kernel_spmd

N_CORES = 8
IN_DIM = 128
OUT_DIM = 512
N_OBS = 25000
M_LANES = 400000
GS = 16                       # lanes per group
M_C = M_LANES // N_CORES      # 50000 lanes per core
G_C = N_OBS // N_CORES        # 3125 groups per core
N_CHUNK = OUT_DIM // 128      # 4 outdim chunks
BLK = 1536                    # lanes per block (3 psum banks)

# --- engine-balance tunables -------------------------------------------------
ACT_COLS = 1120               # evac columns per tile on ACT; rest on DVE
# sum-pool engine per block: PE (accumulating identity matmuls) if
# ib % SUM_MOD < SUM_PE_OF, else a GPSIMD tensor_tensor add-tree.
# (max-pool is DVE-only: the Pool engine rejects TensorTensor max, ACT has
# no reduce, PE can't do max — DVE reduce_max is the only windowed max.)
SUM_MOD = 3
SUM_PE_OF = 2

_compiled = {}


def _blocks(m_c, blk):
    """[(lane0, nlanes)] covering m_c in blk-sized chunks (multiple of GS)."""
    out = []
    s = 0
    while s < m_c:
        out.append((s, min(blk, m_c - s)))
        s += blk
    return out


def _build(m_c: int = M_C) -> bass.Bass:
    nc = bacc.Bacc(None, target_bir_lowering=False)
    f32 = mybir.dt.float32
    bf16 = mybir.dt.bfloat16
    g_c = m_c // GS

    xth_d = nc.dram_tensor("xth", [IN_DIM, m_c], bf16, kind="ExternalInput")
    wth_d = nc.dram_tensor("wth", [IN_DIM, OUT_DIM], bf16, kind="ExternalInput")
    bsc_d = nc.dram_tensor("bsc", [128, N_CHUNK], f32, kind="ExternalInput")
    eye_d = nc.dram_tensor("eye", [128, 128], bf16, kind="ExternalInput")
    omax_d = nc.dram_tensor("omax", [OUT_DIM, g_c], bf16, kind="ExternalOutput")
    osum_d = nc.dram_tensor("osum", [OUT_DIM, g_c], bf16, kind="ExternalOutput")

    blocks = _blocks(m_c, BLK)
    nblk = len(blocks)

    with nc.allow_low_precision("pooled outputs are bf16 by design"), \
            tile.TileContext(nc) as tc:
        with (
            tc.tile_pool(name="singles", bufs=1) as singles,
            tc.tile_pool(name="xin", bufs=3) as xin,
            tc.tile_pool(name="rsb", bufs=4) as rsb,
            tc.tile_pool(name="gtmp", bufs=4) as gtmp,
            tc.tile_pool(name="acc", bufs=1) as accp,
            tc.tile_pool(name="psum_u", bufs=2, space="PSUM") as psum_u,
            tc.tile_pool(name="psum_s", bufs=2, space="PSUM") as psum_s,
        ):
            wth_sb = singles.tile([IN_DIM, OUT_DIM], bf16)
            nc.sync.dma_start(out=wth_sb, in_=wth_d[:, :])
            eye_sb = singles.tile([128, 128], bf16)
            nc.sync.dma_start(out=eye_sb, in_=eye_d[:, :])
            bsc_sb = singles.tile([128, N_CHUNK], f32)
            nc.sync.dma_start(out=bsc_sb, in_=bsc_d[:, :])

            # persistent pooled accumulators [128, chunk, g_c] bf16
            maxp_sb = accp.tile([128, N_CHUNK, g_c], bf16)
            sump_sb = accp.tile([128, N_CHUNK, g_c], bf16)

            # prime the ACT Relu spline table while the first DMA is in flight
            warm_sb = singles.tile([128, 2], f32)
            nc.vector.memset(warm_sb, 0.0)
            nc.scalar.activation(
                out=warm_sb, in_=warm_sb,
                func=mybir.ActivationFunctionType.Relu, bias=0.0, scale=1.0,
            )

            # flush output DMA at these block indices (group ranges completed
            # through the PREVIOUS block, since PE sums lag one block)
            flush_at = {nblk // 4, nblk // 2, (3 * nblk) // 4}
            flush_from = 0

            # deferred PE-sum state for the previous block:
            # (r tile, lb, psum_s tile, g0, gb)
            pending = None

            def emit_pe_sums(p):
                r_sb_p, lb_p, ps_tile, _g0, gbp = p
                for c in range(N_CHUNK):
                    r3p = r_sb_p[0:128, c, 0:lb_p].rearrange(
                        "p (g s) -> p g s", s=GS
                    )
                    for s in range(GS):
                        nc.tensor.matmul(
                            ps_tile[:, c, 0:gbp],
                            eye_sb[:, :],
                            r3p[:, :, s],
                            start=(s == 0), stop=(s == GS - 1),
                        )

            def evac_pe_sums(p):
                _r, _lb, ps_tile, g0p, gbp = p
                # one ACT copy for all 4 chunks: psum fp32 -> bf16 sump
                nc.scalar.activation(
                    out=sump_sb[:, :, g0p : g0p + gbp],
                    in_=ps_tile[:, :, 0:gbp],
                    func=mybir.ActivationFunctionType.Copy,
                    bias=0.0, scale=1.0,
                )

            for ib, (l0, lb) in enumerate(blocks):
                gb = lb // GS
                g0 = l0 // GS

                xth_sb = xin.tile([IN_DIM, BLK], bf16, tag="xth")
                nc.sync.dma_start(out=xth_sb[:, :lb], in_=xth_d[:, l0 : l0 + lb])

                r_sb = rsb.tile([128, N_CHUNK, BLK], bf16, tag="r")

                n_wave = (lb + 511) // 512
                pe_sum_emitted = False
                for c in range(N_CHUNK):
                    enc_ps = psum_u.tile([128, BLK], f32, tag="enc")
                    for w in range(n_wave):
                        w0 = w * 512
                        lw = min(512, lb - w0)
                        nc.tensor.matmul(
                            enc_ps[:, w0 : w0 + lw],
                            wth_sb[:, c * 128 : (c + 1) * 128],
                            xth_sb[:, w0 : w0 + lw],
                            start=True, stop=True,
                        )

                    # deferred PE sums for the previous block, placed after
                    # this block's first two main-chunk waves
                    if c == 2 and pending is not None:
                        emit_pe_sums(pending)
                        pe_sum_emitted = True

                    # r = relu(u + b): ACT takes cols [0:sa], DVE the rest
                    rc = r_sb[:, c, :]
                    sa = min(ACT_COLS, lb)
                    nc.scalar.activation(
                        out=rc[0:128, 0:sa],
                        in_=enc_ps[:, 0:sa],
                        func=mybir.ActivationFunctionType.Relu,
                        bias=bsc_sb[:, c : c + 1],
                        scale=1.0,
                    )
                    if lb > sa:
                        nc.vector.tensor_scalar(
                            out=rc[0:128, sa:lb],
                            in0=enc_ps[:, sa:lb],
                            scalar1=bsc_sb[:, c : c + 1],
                            scalar2=0.0,
                            op0=mybir.AluOpType.add,
                            op1=mybir.AluOpType.max,
                        )

                    r3 = rc[0:128, 0:lb].rearrange("p (g s) -> p g s", s=GS)
                    # max pool: DVE windowed reduce (bf16 packed)
                    nc.vector.reduce_max(
                        out=maxp_sb[:, c, g0 : g0 + gb],
                        in_=r3, axis=mybir.AxisListType.X,
                    )
                    # sum pool for non-PE blocks: gpsimd add-tree
                    # (reads r only in op 1 so the r tile releases quickly)
                    if ib % SUM_MOD >= SUM_PE_OF:
                        t1 = gtmp.tile([128, gb, 8], bf16, tag="t1")
                        nc.gpsimd.tensor_tensor(
                            out=t1, in0=r3[:, :, 0::2], in1=r3[:, :, 1::2],
                            op=mybir.AluOpType.add,
                        )
                        t2 = gtmp.tile([128, gb, 4], bf16, tag="t2")
                        nc.gpsimd.tensor_tensor(
                            out=t2, in0=t1[:, :, 0::2], in1=t1[:, :, 1::2],
                            op=mybir.AluOpType.add,
                        )
                        t3 = gtmp.tile([128, gb, 2], bf16, tag="t3")
                        nc.gpsimd.tensor_tensor(
                            out=t3, in0=t2[:, :, 0::2], in1=t2[:, :, 1::2],
                            op=mybir.AluOpType.add,
                        )
                        nc.gpsimd.tensor_tensor(
                            out=sump_sb[:, c, g0 : g0 + gb],
                            in0=t3[:, :, 0], in1=t3[:, :, 1],
                            op=mybir.AluOpType.add,
                        )

                if pending is not None and not pe_sum_emitted:
                    emit_pe_sums(pending)
                    pe_sum_emitted = True
                if pending is not None:
                    evac_pe_sums(pending)
                    pending = None

                if ib % SUM_MOD < SUM_PE_OF:
                    ps_tile = psum_s.tile([128, N_CHUNK, 128], f32, tag="ps")
                    pending = (r_sb, lb, ps_tile, g0, gb)

                # stream finished group ranges out (sums lag one block)
                if ib in flush_at:
                    done = g0 if pending is not None else g0 + gb
                    if done > flush_from:
                        r0, r1 = flush_from, done
                        flush_from = done
                        for c in range(N_CHUNK):
                            nc.sync.dma_start(
                                out=omax_d[c * 128 : (c + 1) * 128, r0:r1],
                                in_=maxp_sb[:, c, r0:r1],
                            )
                            nc.sync.dma_start(
                                out=osum_d[c * 128 : (c + 1) * 128, r0:r1],
                                in_=sump_sb[:, c, r0:r1],
                            )

            if pending is not None:
                emit_pe_sums(pending)
                evac_pe_sums(pending)
                pending = None

            r0, r1 = flush_from, g_c
            for c in range(N_CHUNK):
                nc.sync.dma_start(
                    out=omax_d[c * 128 : (c + 1) * 128, r0:r1],
                    in_=maxp_sb[:, c, r0:r1],
                )
                nc.sync.dma_start(
                    out=osum_d[c * 128 : (c + 1) * 128, r0:r1],
                    in_=sump_sb[:, c, r0:r1],
                )

    nc.compile()
    return nc


def _get_nc() -> bass.Bass:
    if "k" not in _compiled:
        _compiled["k"] = _build()
    return _compiled["k"]


def _host_prep(lane_encoding, W, b):
    """Returns the per-core in_maps."""
    bf = ml_dtypes.bfloat16
    xT = np.ascontiguousarray(lane_encoding.T)          # [128, M]
    xh = xT.astype(bf)
    wh = np.ascontiguousarray(W.T).astype(bf)           # [128, 512]
    bsc = np.ascontiguousarray(
        b.reshape(N_CHUNK, 128).T.astype(np.float32)    # [128, chunk]
    )
    eye = np.eye(128, dtype=bf)

    in_maps = []
    for c in range(N_CORES):
        sl = slice(c * M_C, (c + 1) * M_C)
        in_maps.append({
            "xth": np.ascontiguousarray(xh[:, sl]),
            "wth": wh, "bsc": bsc, "eye": eye,
        })
    return in_maps


def _run(lane_encoding, W, b, trace: bool = False):
    nc = _get_nc()
    in_maps = _host_prep(lane_encoding, W, b)
    try:
        res = run_bass_kernel_spmd(
            nc, in_maps, core_ids=list(range(N_CORES)), trace=trace
        )
    except Exception:
        # transient NRT_EXEC_UNIT_UNRECOVERABLE wedges have been observed;
        # a single retry usually succeeds
        res = run_bass_kernel_spmd(
            nc, in_maps, core_ids=list(range(N_CORES)), trace=trace
        )
    out = np.empty((N_OBS, 2 * OUT_DIM), dtype=np.float32)
    for c in range(N_CORES):
        gsl = slice(c * G_C, (c + 1) * G_C)
        out[gsl, :OUT_DIM] = res.results[c]["omax"].T.astype(np.float32)
        out[gsl, OUT_DIM:] = res.results[c]["osum"].T.astype(np.float32) / GS
    return out, res


def kernel(obs_encoding, lane_encoding, same_obs_mask, W, b):
    out, _ = _run(
        np.asarray(lane_encoding, dtype=np.float32),
        np.asarray(W, dtype=np.float32),
        np.asarray(b, dtype=np.float32),
    )
    return out
